# revision 39
# baseline (speedup 1.0000x reference)
"""Trainium2 Bass kernel for nn_BasicNCAModel (neural cellular automaton).

Model (per step, 4 steps):
  y = concat([x, dwconv3x3(x, f1), dwconv3x3(x, f2)])   (reflect pad)
  dx = relu(y @ w1 + b1) @ w2
  x  = x + dx * (stoch > 0.5) * ch_mask

Kernel strategy:
  - Pure data parallel: batch 16 -> 2 samples on each of 8 NeuronCores.
  - Channel-major layout [C=32, H, W]; depthwise convs + first dense layer
    fold into a 3x3 conv with effective weights
    W_eff[dy,dx] = diag(f1[dy,dx]) @ w1[32:64] + diag(f2[dy,dx]) @ w1[64:96]
    (+ w1[0:32] at the center tap). K = 3 vertically shifted copies of x
    stacked on partitions; horizontal taps are free-dim AP offsets.
  - Mixed precision: the two outer horizontal taps (only ~15% of the dx
    variance) form the 2 k-tiles of ONE fp8e4 DoubleRow matmul per output
    half (overlapping stride-2 moving AP over an fp8 copy of the band);
    the center tap (K=97 incl. ones row for the bias) and layer 2 run in
    fp16 to keep quantization error ~1e-2. 6 PE passes of 512 moving rows
    per 512-pixel tile (vs 8 all-fp16 passes).
  - The fire mask is precomputed 0/1 fp8 on host [H, W] and replicated 32x
    on device into [H/2, 32, 2, W] via DRAM->DRAM DMAs; its (row-pair,
    channel) -> partition map is affine so one HWDGE DMA loads a whole
    band's mask [32, BR*W] with no on-chip broadcast.
  - Relu+bias splits between the scalar engine (cols [0:RELU_ACT]) and DVE
    (tensor_scalar_max, cols [RELU_ACT:1024]), both writing fp8.
  - Residual: dxm = mask * dx on DVE (PSUM source), then xn = dxm + xc in
    fp16 on DVE (2x mode); relu runs fully on the scalar engine. This
    keeps the SWDGE queue for the xt8 cast-load only (HWDGE carries the
    fp16 state traffic), so neither DMA path saturates.
  - Software pipelining: layer 2 runs at a 2-tile lag behind layer 1 (the
    relu latency never stalls the PE), and band i+1's loads are emitted
    before band i's accum stores so SWDGE prefetch crosses band bounds.
  - State x is fp16 DRAM, column-padded [C, H, W+2] with reflect columns
    stored in the pads (band loads contiguous). Ping-pong across steps.
"""

import numpy as np
import ml_dtypes
from contextlib import ExitStack

import concourse.bacc as bacc
import concourse.tile as tile
from concourse import mybir
from concourse.ap import AP
from concourse.bass_utils import run_bass_kernel_spmd

F32 = mybir.dt.float32
F16 = mybir.dt.float16
F8 = mybir.dt.float8e4
AF = mybir.ActivationFunctionType
OP = mybir.AluOpType
DRMODE = mybir.MatmulPerfMode.DoubleRow

B, C, H, W = 16, 32, 256, 256
IMG = 3
FIRE = 0.5
NCORES = 8
BPC = B // NCORES          # samples per core = 2
BR = 16                    # band rows
NB = H // BR               # bands per sample = 16
ROWS_PER_TILE = 2          # 2 rows x 256 cols = 512-pixel matmul tiles
TPB = BR // ROWS_PER_TILE  # tiles per band = 8
NSTEP = 4
WP = W + 2                 # padded row length 258

RELU_ACT = 1024            # relu split: scalar engine cols [0:RELU_ACT]


def _seg_rows(r0: int, dy: int):
    """Contiguous (src_row, dst_row, n) segments for one vertical copy,
    with reflect handling at the image top/bottom (reflect: -1->1, 256->254)."""
    rows = [r0 + dy + i for i in range(BR)]
    refl = [(-r if r < 0 else (2 * (H - 1) - r if r > H - 1 else r)) for r in rows]
    segs = []
    i = 0
    while i < BR:
        j = i + 1
        while j < BR and refl[j] == refl[i] + (j - i):
            j += 1
        segs.append((refl[i], i, j - i))
        i = j
    return segs


def _build():
    nc = bacc.Bacc("TRN2", target_bir_lowering=False, debug=False,
                   num_devices=NCORES)
    xin = nc.dram_tensor("xin", [BPC, C, H, WP], F16, kind="ExternalInput").ap()
    mask8 = nc.dram_tensor("mask8", [NSTEP, BPC, H, W], F8,
                           kind="ExternalInput").ap()
    wmp = nc.dram_tensor("wmp", [96, 512], F8, kind="ExternalInput").ap()
    w0 = nc.dram_tensor("w0", [97, 256], F16, kind="ExternalInput").ap()
    w2h = nc.dram_tensor("w2h", [128, 64], F16, kind="ExternalInput").ap()
    yout = nc.dram_tensor("y", [BPC, C, H, WP], F16, kind="ExternalOutput").ap()

    with tile.TileContext(nc) as tc, ExitStack() as ctx:
        dram = ctx.enter_context(tc.tile_pool(name="dram", bufs=1, space="DRAM"))
        xA = dram.tile([BPC, C, H, WP], F16, name="xA")
        xB = dram.tile([BPC, C, H, WP], F16, name="xB")
        # mask replicated 32x: [step, s, row-pair, channel-copy, 2, W]
        mrep = dram.tile([NSTEP, BPC, H // 2, 32, 2, W], F8, name="mrep")

        # ---- replicate the compact mask to all 32 channel slots by
        # log-doubling (6 DMAs per (step, sample) instead of 32) ----
        for step in range(NSTEP):
            for s in range(BPC):
                msrc = mask8[step, s].rearrange("(p two) w -> p two w", two=2)
                nc.sync.dma_start(mrep[step, s, :, 0], msrc)
                n = 1
                while n < 32:
                    nc.sync.dma_start(
                        mrep[step, s, :, n:2 * n]
                        .rearrange("p c two w -> p c (two w)"),
                        mrep[step, s, :, 0:n]
                        .rearrange("p c two w -> p c (two w)"))
                    n *= 2

        wpool = ctx.enter_context(tc.tile_pool(name="wpool", bufs=1))
        wmpt = wpool.tile([96, 512], F8, name="wmpt")
        w0t = wpool.tile([97, 256], F16, name="w0t")
        w2t = wpool.tile([128, 64], F16, name="w2t")
        nc.sync.dma_start(wmpt[:], wmp)
        nc.sync.dma_start(w0t[:], w0)
        nc.sync.dma_start(w2t[:], w2h)

        xt_pool = ctx.enter_context(tc.tile_pool(name="xt", bufs=1))
        ms_pool = ctx.enter_context(tc.tile_pool(name="ms", bufs=4))
        dxm_pool = ctx.enter_context(tc.tile_pool(name="dxm", bufs=3))
        hs_pool = ctx.enter_context(tc.tile_pool(name="hs", bufs=3))
        hp_pool = ctx.enter_context(tc.tile_pool(name="hp", bufs=3, space="PSUM"))
        dxp_pool = ctx.enter_context(tc.tile_pool(name="dxp", bufs=2, space="PSUM"))
        xc_pool = ctx.enter_context(tc.tile_pool(name="xc", bufs=4))
        xn_pool = ctx.enter_context(tc.tile_pool(name="xn", bufs=3))

        # fp8 copy of the band for the DoubleRow outer-tap pass
        xt8_pool = ctx.enter_context(tc.tile_pool(name="xt8", bufs=4))
        # manual 4-buffer rotation for the fp16 copy so the ones row (bias)
        # is primed once per buffer instead of per band
        xts = [xt_pool.tile([97, BR * WP], F16, name=f"xt{i}")
               for i in range(4)]
        for xt in xts:
            nc.gpsimd.memset(xt[96:97, :], 1.0)

        srcs = [xin, xA[:], xB[:], xA[:]]
        dsts = [xA[:], xB[:], xA[:], yout]
        bands = [(step, s, b) for step in range(NSTEP)
                 for s in range(BPC) for b in range(NB)]
        state = {}  # band index -> dict of live tiles

        def emit_loads(i):
            step, s, b = bands[i]
            src, dst = srcs[step], dsts[step]
            r0 = b * BR
            dst_band = dst[s, :, r0:r0 + BR, :]
            # ---- load: 3 vertically shifted copies of the band.
            # partition groups: 0-31 dy=0, 32-63 dy=-1, 64-95 dy=+1.
            # fp16 copy via HWDGE (center tap); fp8 copy via SWDGE cast in
            # flight (DoubleRow outer taps). Reflect columns are already
            # stored in the DRAM pads.
            xt = xts[i % 4]
            xtr = xt[:].rearrange("p (r c) -> p r c", c=WP)
            xt8 = xt8_pool.tile([96, BR * WP], F8)
            xt8r = xt8[:].rearrange("p (r c) -> p r c", c=WP)
            for gi, dy in enumerate((0, -1, 1)):
                p0 = gi * 32
                for (sr, dr, n) in _seg_rows(r0, dy):
                    nc.sync.dma_start(xtr[p0:p0 + 32, dr:dr + n, :],
                                      src[s, :, sr:sr + n, :])
                    nc.gpsimd.dma_start(xt8r[p0:p0 + 32, dr:dr + n, :],
                                        src[s, :, sr:sr + n, :])
            # ---- band fire mask [32, BR*W] via one affine DMA ----
            ms = ms_pool.tile([32, BR * W], F8)
            rp0 = r0 // 2
            nc.sync.dma_start(ms[:], mrep[step, s, rp0:rp0 + TPB]
                              .rearrange("a b c d -> b a (c d)"))
            # fp16 copy of the band for the residual add
            xc = xc_pool.tile([32, BR * WP], F16)
            nc.sync.dma_start(xc[:], src[s, :, r0:r0 + BR, :]
                              .rearrange("p r c -> p (r c)"))
            state[i] = dict(xt=xt, xtr=xtr, xt8=xt8, ms=ms, xc=xc,
                            dst_band=dst_band)

        def emit_l1(i, t):
            """Layer 1 (4 passes) + relu for tile t of band i."""
            st = state[i]
            xtr, xt8 = st["xtr"], st["xt8"]
            xbase = xt8[:]
            pstride = xbase.ap[0][0]
            rt = t * ROWS_PER_TILE
            hp = hp_pool.tile([128, 1024], F32)
            rhs_mp = AP(tensor=xbase.tensor,
                        offset=xbase.offset + rt * WP,
                        ap=[[pstride, 96], [2, 2],
                            [WP, ROWS_PER_TILE], [1, W]])
            for h in range(2):
                out = hp[:, h * 512:(h + 1) * 512]
                lhs_mp = (wmpt[:, h * 256:(h + 1) * 256]
                          .rearrange("p (i m) -> p i m", i=2))
                nc.tensor.matmul(out, lhs_mp, rhs_mp, start=True, stop=False,
                                 perf_mode=DRMODE)
                nc.tensor.matmul(out, w0t[:, h * 128:(h + 1) * 128],
                                 xtr[0:97, rt:rt + ROWS_PER_TILE, 1:W + 1],
                                 start=False, stop=True)
            # ---- relu (bias via ones row) -> fp16 ----
            hs = hs_pool.tile([128, 1024], F16)
            nc.scalar.activation(hs[:, 0:RELU_ACT], hp[:, 0:RELU_ACT],
                                 AF.Relu)
            if RELU_ACT < 1024:
                nc.vector.tensor_scalar_max(hs[:, RELU_ACT:1024],
                                            hp[:, RELU_ACT:1024], 0.0)
            return hs

        def emit_finalize(i, t, hs_p):
            """Layer 2 (2 passes) + masked residual for tile t of band i;
            on the band's last tile: reflect pads + store."""
            st = state[i]
            dxp = dxp_pool.tile([32, 512], F32)
            nc.tensor.matmul(dxp[:], w2t[:, 0:32], hs_p[:, 0:512],
                             start=True, stop=False)
            nc.tensor.matmul(dxp[:], w2t[:, 32:64], hs_p[:, 512:1024],
                             start=False, stop=True)
            # ---- dxm = mask * dx (fp16) ----
            csl = slice(t * 512, (t + 1) * 512)
            nc.vector.tensor_tensor(st["dxm"][:, csl], st["ms"][:, csl],
                                    dxp[:], op=OP.mult)
            # ---- residual add xn = dxm + xc (fp16, DVE 2x) ----
            rp = t * ROWS_PER_TILE
            xnr, xcr = st["xnr"], st["xcr"]
            nc.vector.tensor_add(
                xnr[:, rp:rp + ROWS_PER_TILE, 1:W + 1],
                st["dxm"][:, csl].rearrange("p (r c) -> p r c", c=W),
                xcr[:, rp:rp + ROWS_PER_TILE, 1:W + 1])
            if t == TPB - 1:
                # reflect pads then store the fp16 band (HWDGE)
                nc.vector.tensor_copy(xnr[:, :, 0:1], xnr[:, :, 2:3])
                nc.vector.tensor_copy(xnr[:, :, WP - 1:WP],
                                      xnr[:, :, WP - 3:WP - 2])
                nc.sync.dma_start(
                    st["dst_band"].rearrange("p r c -> p (r c)"),
                    st["xn"][:])
                state.pop(i)

        # global tile pipeline: layer 2 runs at a constant 2-tile lag
        # behind layer 1 ACROSS band boundaries (no per-band PE burst), and
        # band loads prefetch 2 bands ahead of compute
        tiles = [(i, t) for i in range(len(bands)) for t in range(TPB)]
        emit_loads(0)
        emit_loads(1)
        pend = []
        for g, (i, t) in enumerate(tiles):
            if t == 0:
                # entering band i: prefetch band i+2's inputs and allocate
                # band i's compute-side tiles
                if i + 2 < len(bands):
                    emit_loads(i + 2)
                st = state[i]
                st["dxm"] = dxm_pool.tile([32, BR * W], F16, name="dxm")
                xn = xn_pool.tile([32, BR * WP], F16, name="xn")
                st["xn"] = xn
                st["xnr"] = xn[:].rearrange("p (r c) -> p r c", c=WP)
                st["xcr"] = st["xc"][:].rearrange("p (r c) -> p r c", c=WP)
            pend.append((i, t, emit_l1(i, t)))
            if len(pend) > 2:
                fi, ft, fhs = pend.pop(0)
                emit_finalize(fi, ft, fhs)
        for fi, ft, fhs in pend:
            emit_finalize(fi, ft, fhs)
    nc.compile()
    return nc


_NC_CACHE = None


def _get_nc():
    global _NC_CACHE
    if _NC_CACHE is None:
        _NC_CACHE = _build()
    return _NC_CACHE


def _make_in_maps(x, f1, f2, w1, b1, w2, stoch):
    f1 = np.asarray(f1, np.float64)[:, :, 0, :]   # [3,3,32]
    f2 = np.asarray(f2, np.float64)[:, :, 0, :]
    w1 = np.asarray(w1, np.float64)               # [96,256]
    b1 = np.asarray(b1, np.float64)               # [256]
    w2 = np.asarray(w2, np.float64).copy()        # [256,32]
    w2[:, :IMG] = 0.0                             # ch_mask folded into w2

    # W_eff[dy,dx][c,:] = f1[dy,dx,c]*w1[32+c,:] + f2[dy,dx,c]*w1[64+c,:]
    #                     (+ w1[c,:] at the center tap)
    weff = (f1[:, :, :, None] * w1[None, None, 32:64, :]
            + f2[:, :, :, None] * w1[None, None, 64:96, :])   # [3,3,32,256]
    weff[1, 1] += w1[0:32, :]

    def col(dxi):  # stack the 3 vertical taps along K for horizontal tap dxi
        # row order matches xt partition groups: dy=0, dy=-1, dy=+1
        return np.concatenate([weff[1, dxi], weff[0, dxi], weff[2, dxi]], axis=0)

    F8NP = ml_dtypes.float8_e4m3
    wm, wpm = col(0), col(2)
    # DR stationary per half: [wm_half | wp_half] along the free dim
    wmp = np.concatenate([wm[:, 0:128], wpm[:, 0:128],
                          wm[:, 128:256], wpm[:, 128:256]], axis=1).astype(F8NP)
    w0 = np.concatenate([col(1), b1[None, :]], axis=0).astype(np.float16)
    w2h = np.concatenate([w2[0:128, :], w2[128:256, :]],
                         axis=1).astype(np.float16)

    x = np.asarray(x, np.float32)
    m8 = (np.asarray(stoch, np.float64) > FIRE).astype(F8NP)
    in_maps = []
    for i in range(NCORES):
        xi = np.transpose(x[i * BPC:(i + 1) * BPC], (0, 3, 1, 2))  # [2,32,H,W]
        xpad = np.zeros((BPC, C, H, WP), np.float16)
        xpad[:, :, :, 1:W + 1] = xi
        xpad[:, :, :, 0] = xi[:, :, :, 1]        # reflect col pads
        xpad[:, :, :, WP - 1] = xi[:, :, :, W - 2]
        mi = np.ascontiguousarray(m8[:, i * BPC:(i + 1) * BPC, :, :, 0])
        in_maps.append({"xin": xpad, "mask8": mi, "wmp": wmp, "w0": w0,
                        "w2h": w2h})
    return in_maps


def kernel(x, f1, f2, w1, b1, w2, stoch, steps):
    assert int(steps) == NSTEP, f"kernel compiled for {NSTEP} steps, got {steps}"
    nc = _get_nc()
    in_maps = _make_in_maps(x, f1, f2, w1, b1, w2, stoch)
    res = run_bass_kernel_spmd(nc, in_maps, core_ids=list(range(NCORES)))
    outs = []
    for i in range(NCORES):
        yi = res.results[i]["y"][:, :, :, 1:W + 1]     # strip col pads
        outs.append(np.transpose(yi, (0, 2, 3, 1)))    # -> [2,256,256,32]
    return np.ascontiguousarray(np.concatenate(outs, axis=0)).astype(np.float32)
